# revision 2
# baseline (speedup 1.0000x reference)
"""BigBird ViT forward on 8 Trainium2 NeuronCores.

Sharding: 2 groups of 4 cores (one per batch element). Within a group,
tokens are sharded 4-way (272 of the 1088 padded tokens per core) for all
dense matmuls / layernorms (weights replicated, streamed from HBM in bf16),
and attention is computed for the core's own 272 query tokens over all 12
heads, after a per-layer AllGather of K^T and V (one fused collective).

Everything on-chip lives transposed ([feature, token]) so the PE contracts
over partitions without any activation transposes; LayerNorm reductions over
the feature dim use ones-vector matmuls (float32r) on the PE.

The BigBird band/random/global structure (plus seq padding) is applied as a
multiplicative {0,1} bf16 mask on the unnormalized attention probabilities;
with S=1025 the reference's -10000 additive masking underflows exp() to
exactly 0, so this is an exact reformulation.
"""
import os
import sys

sys.path.insert(0, "/opt/trn_rl_repo")

import numpy as np
import ml_dtypes

import concourse.bass as bass
import concourse.bacc as bacc
import concourse.mybir as mybir
import concourse.tile as tile
from concourse.bass_utils import run_bass_kernel_spmd

F32 = mybir.dt.float32
F32R = mybir.dt.float32r
BF16 = mybir.dt.bfloat16
AF = mybir.ActivationFunctionType
ALU = mybir.AluOpType
BF = ml_dtypes.bfloat16

# model dims
BS = 64; NH = 12; HD = 64; D = 768; F = 3072; L = 12; R = 3
SEQ = 1025
SEQP = 1088           # padded to 17 blocks of 64
NBLK = 17
T = SEQP // 4         # tokens per core = 272
DT = D // 128         # 6 feature tiles
FT = F // 128         # 24 ffn tiles
KT = 9                # k tiles over 1152 (1088 padded up; tile 8 is half real)
KPAD = 1152           # k range padded to 9*128
VCOLS = NH * (HD + 1)  # 780: per-head [64 V cols + 1 ones col]
SC = 1.0 / np.sqrt(HD)

NLAYERS = int(os.environ.get("BB_NLAYERS", str(L)))

_CACHE = {}


# ---------------------------------------------------------------- builder

def build_program(nlayers=NLAYERS):
    nc = bacc.Bacc("TRN2", target_bir_lowering=False, debug=False, num_devices=8)

    # ---- DRAM I/O -------------------------------------------------------
    pe_in = nc.dram_tensor("pe_in", [128, DT * T], BF16, kind="ExternalInput")
    add_in = nc.dram_tensor("add_in", [128, DT * T], F32, kind="ExternalInput")
    mask_in = nc.dram_tensor("mask_in", [128, NH * (KT - 1) * T], BF16, kind="ExternalInput")
    pw_in = nc.dram_tensor("pw", [D, D], BF16, kind="ExternalInput")
    normp_in = nc.dram_tensor("normp", [128, 2 * DT], F32, kind="ExternalInput")
    wq = [nc.dram_tensor(f"wq{i}", [D, D], BF16, kind="ExternalInput") for i in range(nlayers)]
    wk = [nc.dram_tensor(f"wk{i}", [D, D], BF16, kind="ExternalInput") for i in range(nlayers)]
    wv = [nc.dram_tensor(f"wv{i}", [D, VCOLS], BF16, kind="ExternalInput") for i in range(nlayers)]
    wo = [nc.dram_tensor(f"wo{i}", [D, D], BF16, kind="ExternalInput") for i in range(nlayers)]
    # w1/w2 shipped pre-tiled o-major: [128, OT*CT*128] with each 128x128 tile
    # contiguous, all contraction tiles of one output tile adjacent.
    w1 = [nc.dram_tensor(f"w1{i}", [128, FT * D], BF16, kind="ExternalInput") for i in range(nlayers)]
    w2 = [nc.dram_tensor(f"w2{i}", [128, DT * F], BF16, kind="ExternalInput") for i in range(nlayers)]
    lnp = [nc.dram_tensor(f"lnp{i}", [128, 72], F32, kind="ExternalInput") for i in range(nlayers)]
    bvb = [nc.dram_tensor(f"bvb{i}", [128, VCOLS], BF16, kind="ExternalInput") for i in range(nlayers)]
    out_t = nc.dram_tensor("out", [128, DT * T], F32, kind="ExternalOutput")

    # collective bounce buffers (internal DRAM), reused across layers
    KV_K = D * T              # K^T-local elems
    KV_V = T * VCOLS          # V-nat-local elems
    KV = KV_K + KV_V
    kv_in = nc.dram_tensor("kv_in", [KV], BF16)
    kv_out = nc.dram_tensor("kv_out", [4 * KV], BF16)

    with tile.TileContext(nc) as tc:
        # ---- persistent SBUF tensors -----------------------------------
        X = nc.alloc_sbuf_tensor("X", [128, DT * T], F32)          # residual, ft-tile major
        xn = nc.alloc_sbuf_tensor("xn", [128, DT * T], BF16)       # LN output
        Qt = nc.alloc_sbuf_tensor("Qt", [128, DT * T], BF16)       # Q^T local
        Ktl = nc.alloc_sbuf_tensor("Ktl", [128, DT * T], BF16)     # K^T local
        Vnl = nc.alloc_sbuf_tensor("Vnl", [128, 3 * VCOLS], BF16)  # V-nat local (3 tok tiles)
        Ktf = nc.alloc_sbuf_tensor("Ktf", [128, DT * KPAD], BF16)  # K^T full (padded to 1152)
        Vnf = nc.alloc_sbuf_tensor("Vnf", [128, KT * VCOLS], BF16)  # V-nat full
        ctx = nc.alloc_sbuf_tensor("ctx", [128, DT * T], BF16)     # attention out^T
        hsb = nc.alloc_sbuf_tensor("hsb", [128, FT * T], BF16)     # ffn hidden^T
        msk = nc.alloc_sbuf_tensor("msk", [128, NH * (KT - 1) * T], BF16)
        m8c = nc.alloc_sbuf_tensor("m8c", [128, T], BF16)
        onesb = nc.alloc_sbuf_tensor("onesb", [128, 1], F32)       # for LN stats (as f32r)
        ones1 = nc.alloc_sbuf_tensor("ones1", [1, 128], F32)       # for bcast (Kc=1)
        lnp_sb = nc.alloc_sbuf_tensor("lnp_sb", [128, 72], F32)
        bvb_sb = nc.alloc_sbuf_tensor("bvb_sb", [128, VCOLS], BF16)
        mu_b = nc.alloc_sbuf_tensor("mu_b", [128, T], F32)
        rs_b = nc.alloc_sbuf_tensor("rs_b", [128, T], F32)
        epsb = nc.alloc_sbuf_tensor("epsb", [128, 1], F32)

        with (
            tc.tile_pool(name="wpool", bufs=2) as wpool,        # weight slabs [128, <=780]
            tc.tile_pool(name="w1pool", bufs=2) as w1pool,      # ffn slabs
            tc.tile_pool(name="work", bufs=2) as work,
            tc.tile_pool(name="stat", bufs=4) as stat,
            tc.tile_pool(name="ppool", bufs=2) as ppool,        # P tiles per head
            tc.tile_pool(name="ps", bufs=3, space="PSUM") as ps,
            tc.tile_pool(name="psc", bufs=2, space="PSUM") as psc,
            tc.tile_pool(name="pss", bufs=3, space="PSUM") as pss,
        ):
            nc.vector.memset(onesb[:], 1.0)
            nc.vector.memset(epsb[:], 1e-5)
            nc.vector.memset(ones1[:], 1.0)
            # zero the k-padding region of Ktf (cols 1088..1151 of each ft tile)
            for t in range(DT):
                nc.vector.memset(Ktf[:, t * KPAD + SEQP: (t + 1) * KPAD], 0.0)
            # zero lower half of last Vn tile (tokens 1088..1151 don't exist)
            nc.vector.memset(Vnf[64:128, (KT - 1) * VCOLS: KT * VCOLS], 0.0)
            # load masks (resident)
            nc.sync.dma_start(out=msk[:], in_=mask_in[:, :])
            # constant mask for k-tile 8: only key 1024 (row 0) is real
            nc.vector.memset(m8c[:], 0.0)
            nc.vector.memset(m8c[0:1, :], 1.0)

            def ln_params(col0):
                g = lnp_sb[:, col0:col0 + DT]
                b = lnp_sb[:, col0 + DT:col0 + 2 * DT]
                return g, b

            def layernorm(src_f32, gcol, out_bf, scale_cols=None):
                """src [128, DT*T] f32 ft-major -> out bf16, LN over features."""
                # sum and sumsq via ones-matmul (f32r) accumulated over DT tiles
                sum_ps = pss.tile([1, T], F32, tag="st")
                sq_ps = pss.tile([1, T], F32, tag="st")
                sq = work.tile([128, T], F32, tag="lnsq")
                for t in range(DT):
                    s = src_f32[:, t * T:(t + 1) * T]
                    nc.tensor.matmul(sum_ps[:], onesb[:],
                                     s, start=(t == 0), stop=(t == DT - 1))
                for t in range(DT):
                    s = src_f32[:, t * T:(t + 1) * T]
                    nc.vector.tensor_mul(sq[:], s, s)
                    nc.tensor.matmul(sq_ps[:], onesb[:],
                                     sq[:], start=(t == 0), stop=(t == DT - 1))
                mu = stat.tile([1, T], F32, tag="mu")
                var = stat.tile([1, T], F32, tag="var")
                rstd = stat.tile([1, T], F32, tag="rstd")
                nc.scalar.activation(mu[:], sum_ps[:], AF.Identity, scale=1.0 / D)
                nc.scalar.activation(var[:], sq_ps[:], AF.Identity, scale=1.0 / D)
                mu2 = stat.tile([1, T], F32, tag="mu2")
                nc.vector.tensor_mul(mu2[:], mu[:], mu[:])
                nc.vector.tensor_sub(var[:], var[:], mu2[:])
                # rstd = 1/sqrt(var + eps)
                nc.scalar.activation(rstd[:], var[:], AF.Sqrt, bias=epsb[0:1, 0:1])
                nc.vector.reciprocal(rstd[:], rstd[:])
                # broadcast mu, rstd to [128, T] via Kc=1 matmul
                mu_ps = pss.tile([128, T], F32, tag="st")
                nc.tensor.matmul(mu_ps[:], ones1[:],
                                 mu[:], start=True, stop=True)
                nc.scalar.copy(mu_b[:], mu_ps[:])
                rs_ps = pss.tile([128, T], F32, tag="st")
                nc.tensor.matmul(rs_ps[:], ones1[:],
                                 rstd[:], start=True, stop=True)
                nc.scalar.copy(rs_b[:], rs_ps[:])
                g, b = ln_params(gcol)
                for t in range(DT):
                    tmp = work.tile([128, T], F32, tag="lntmp")
                    nc.vector.tensor_sub(tmp[:], src_f32[:, t * T:(t + 1) * T], mu_b[:])
                    nc.vector.tensor_mul(tmp[:], tmp[:], rs_b[:])
                    nc.vector.tensor_scalar(
                        out_bf[:, t * T:(t + 1) * T], tmp[:],
                        g[:, t:t + 1], b[:, t:t + 1], op0=ALU.mult, op1=ALU.add)

            def proj_t2(wdram, src_bf, out_bf, bias_col=None, ncols=D):
                """out^T = w^T @ src with psum accumulation over contraction tiles.
                Loop order: for each output tile, accumulate over D tiles."""
                OT = ncols // 128
                slabs = []
                for t in range(DT):
                    slab = wpool.tile([128, ncols], BF16, tag=f"wslab{t % 3}")
                    nc.sync.dma_start(out=slab[:], in_=wdram[t * 128:(t + 1) * 128, :])
                    slabs.append(slab)
                for o in range(OT):
                    psm = ps.tile([128, T], F32, tag="mm")
                    for t in range(DT):
                        nc.tensor.matmul(psm[:], slabs[t][:, o * 128:(o + 1) * 128],
                                         src_bf[:, t * T:(t + 1) * T],
                                         start=(t == 0), stop=(t == DT - 1))
                    if bias_col is not None:
                        nc.scalar.activation(
                            out_bf[:, o * T:(o + 1) * T], psm[:], AF.Identity,
                            bias=lnp_sb[:, bias_col + o:bias_col + o + 1])
                    else:
                        nc.scalar.copy(out_bf[:, o * T:(o + 1) * T], psm[:])

            # ---- embedding -------------------------------------------------
            # X^T = pw^T @ pein + add, streaming pe/add chunks from DRAM
            slabs = []
            for t in range(DT):
                slab = wpool.tile([128, D], BF16, tag=f"wslab{t % 3}")
                nc.sync.dma_start(out=slab[:], in_=pw_in[t * 128:(t + 1) * 128, :])
                slabs.append(slab)
            for o in range(DT):
                psm = ps.tile([128, T], F32, tag="mm")
                for t in range(DT):
                    peint = work.tile([128, T], BF16, tag="peint")
                    nc.sync.dma_start(out=peint[:],
                                      in_=pe_in[:, t * T:(t + 1) * T])
                    nc.tensor.matmul(psm[:], slabs[t][:, o * 128:(o + 1) * 128],
                                     peint[:], start=(t == 0), stop=(t == DT - 1))
                addt = work.tile([128, T], F32, tag="wores")
                nc.sync.dma_start(out=addt[:], in_=add_in[:, o * T:(o + 1) * T])
                nc.vector.tensor_add(X[:, o * T:(o + 1) * T], psm[:], addt[:])

            # ---- layers ----------------------------------------------------
            for i in range(nlayers):
                nc.sync.dma_start(out=lnp_sb[:], in_=lnp[i][:, :])
                nc.sync.dma_start(out=bvb_sb[:], in_=bvb[i][:, :])

                # LN1
                layernorm(X, 0, xn)

                # local projections
                proj_t2(wq[i], xn, Qt, bias_col=24)
                proj_t2(wk[i], xn, Ktl, bias_col=30)

                # V natural: lhsT = xn tiles [128D, tokcols], rhs = wv slab [128D, 780]
                vslabs = []
                for t in range(DT):
                    slab = wpool.tile([128, VCOLS], BF16, tag=f"wslab{t % 3}")
                    nc.sync.dma_start(out=slab[:], in_=wv[i][t * 128:(t + 1) * 128, :])
                    vslabs.append(slab)
                for m in range(3):  # token tiles 128,128,16
                    rows = 128 if m < 2 else T - 256
                    for half in range(2):  # n chunks of 390
                        n0, n1 = half * 390, (half + 1) * 390
                        psm = ps.tile([128, 390], F32, tag="mm")
                        for t in range(DT):
                            nc.tensor.matmul(
                                psm[:rows, :], xn[:, t * T + m * 128: t * T + m * 128 + rows],
                                vslabs[t][:, n0:n1], start=(t == 0), stop=(t == DT - 1))
                        nc.vector.tensor_add(
                            Vnl[:rows, m * VCOLS + n0: m * VCOLS + n1],
                            psm[:rows, :], bvb_sb[:rows, n0:n1])

                # bounce to DRAM:  K^T [D, T] then V [T, VCOLS]
                kin2 = kv_in[:].rearrange("(a b) -> a b", b=T)        # [768+?, T] view of K part
                for t in range(DT):
                    nc.sync.dma_start(
                        out=kin2[t * 128:(t + 1) * 128, :],
                        in_=Ktl[:, t * T:(t + 1) * T])
                vin2 = kv_in[KV_K:].rearrange("(a b) -> a b", b=VCOLS)  # [T, 780]
                for m in range(3):
                    rows = 128 if m < 2 else T - 256
                    nc.sync.dma_start(
                        out=vin2[m * 128:m * 128 + rows, :],
                        in_=Vnl[:rows, m * VCOLS:(m + 1) * VCOLS])

                nc.gpsimd.collective_compute(
                    "AllGather", ALU.bypass,
                    replica_groups=[[0, 1, 2, 3], [4, 5, 6, 7]],
                    ins=[kv_in[:].opt()],
                    outs=[kv_out[:].opt()],
                )

                # assemble K^T full and V full from kv_out
                for c in range(4):
                    kc = kv_out[c * KV: c * KV + KV_K].rearrange("(a b) -> a b", b=T)
                    for t in range(DT):
                        nc.sync.dma_start(
                            out=Ktf[:, t * KPAD + c * T: t * KPAD + (c + 1) * T],
                            in_=kc[t * 128:(t + 1) * 128, :])
                # V rows are contiguous within each chunk; tile over 128-rows
                vfull = []  # (m, rows)
                for m in range(KT):
                    r0 = m * 128
                    rows = 128 if m < KT - 1 else SEQP - r0
                    # may cross one chunk boundary
                    spans = []
                    r = r0
                    while r < r0 + rows:
                        c = r // T
                        take = min((c + 1) * T, r0 + rows) - r
                        spans.append((r, c, take))
                        r += take
                    for (rs, c, take) in spans:
                        vc = kv_out[c * KV + KV_K + (rs - c * T) * VCOLS:
                                    c * KV + KV_K + (rs - c * T + take) * VCOLS]
                        nc.sync.dma_start(
                            out=Vnf[rs - r0: rs - r0 + take, m * VCOLS:(m + 1) * VCOLS],
                            in_=vc.rearrange("(a b) -> a b", b=VCOLS))

                # ---- attention, head by head ----
                for h in range(NH):
                    ft, row = h // 2, (h % 2) * 64
                    qh = Qt[row:row + 64, ft * T:(ft + 1) * T]
                    P = ppool.tile([128, KT * T], BF16, tag="P")
                    for m in range(KT):
                        kh = Ktf[row:row + 64, ft * KPAD + m * 128: ft * KPAD + (m + 1) * 128]
                        psm = ps.tile([128, T], F32, tag="mm")
                        nc.tensor.matmul(psm[:], kh, qh, start=True, stop=True)
                        nc.scalar.activation(P[:, m * T:(m + 1) * T], psm[:], AF.Exp,
                                             scale=float(SC))
                    # mask: k-tiles 0..7 from the loaded mask, tile 8 constant
                    nc.vector.tensor_mul(P[:, 0:(KT - 1) * T], P[:, 0:(KT - 1) * T],
                                         msk[:, h * (KT - 1) * T:(h + 1) * (KT - 1) * T])
                    nc.vector.tensor_mul(P[:, (KT - 1) * T:KT * T],
                                         P[:, (KT - 1) * T:KT * T], m8c[:])
                    # PV + Z (ones col) accumulated over k tiles
                    cps = psc.tile([65, T], F32, tag="ctx")
                    for m in range(KT):
                        vh = Vnf[:, m * VCOLS + h * 65: m * VCOLS + (h + 1) * 65]
                        nc.tensor.matmul(cps[:], vh, P[:, m * T:(m + 1) * T],
                                         start=(m == 0), stop=(m == KT - 1))
                    # divide by Z and store into ctx rows
                    zrec = stat.tile([1, T], F32, tag="zrec")
                    nc.vector.reciprocal(zrec[:], cps[64:65, :])
                    zb = pss.tile([64, T], F32, tag="st")
                    nc.tensor.matmul(zb[:], ones1[:, 0:64],
                                     zrec[:], start=True, stop=True)
                    zbs = work.tile([64, T], F32, tag="zbs")
                    nc.scalar.copy(zbs[:], zb[:])
                    nc.vector.tensor_mul(ctx[row:row + 64, ft * T:(ft + 1) * T],
                                         cps[0:64, :], zbs[:])

                # ---- Wo + residual ----
                oslabs = []
                for t in range(DT):
                    slab = wpool.tile([128, D], BF16, tag=f"wslab{t % 3}")
                    nc.sync.dma_start(out=slab[:], in_=wo[i][t * 128:(t + 1) * 128, :])
                    oslabs.append(slab)
                for o in range(DT):
                    psm = ps.tile([128, T], F32, tag="mm")
                    for t in range(DT):
                        nc.tensor.matmul(psm[:], oslabs[t][:, o * 128:(o + 1) * 128],
                                         ctx[:, t * T:(t + 1) * T],
                                         start=(t == 0), stop=(t == DT - 1))
                    tmp = work.tile([128, T], F32, tag="wores")
                    nc.scalar.activation(tmp[:], psm[:], AF.Identity,
                                         bias=lnp_sb[:, 36 + o:37 + o])
                    nc.vector.tensor_add(X[:, o * T:(o + 1) * T],
                                         X[:, o * T:(o + 1) * T], tmp[:])

                # LN2 -> xn (reuse buffer)
                layernorm(X, 12, xn)

                # ---- FFN ----
                # w1 is host-tiled o-major: cols (o*DT + t)*128 hold W1-tile (t, o)
                for o in range(FT):
                    slab = w1pool.tile([128, D], BF16, tag="w1o")
                    nc.sync.dma_start(out=slab[:], in_=w1[i][:, o * D:(o + 1) * D])
                    psm = ps.tile([128, T], F32, tag="mm")
                    for t in range(DT):
                        nc.tensor.matmul(psm[:], slab[:, t * 128:(t + 1) * 128],
                                         xn[:, t * T:(t + 1) * T],
                                         start=(t == 0), stop=(t == DT - 1))
                    nc.scalar.activation(hsb[:, o * T:(o + 1) * T], psm[:], AF.Gelu,
                                         bias=lnp_sb[:, 48 + o:49 + o])
                # w2 host-tiled o-major: cols (o*FT + t)*128 hold W2-tile (t, o)
                for o in range(DT):
                    slab = w1pool.tile([128, F], BF16, tag="w2o")
                    nc.sync.dma_start(out=slab[:], in_=w2[i][:, o * F:(o + 1) * F])
                    psm = ps.tile([128, T], F32, tag="mm")
                    for t in range(FT):
                        nc.tensor.matmul(psm[:], slab[:, t * 128:(t + 1) * 128],
                                         hsb[:, t * T:(t + 1) * T],
                                         start=(t == 0), stop=(t == FT - 1))
                    tmp = work.tile([128, T], F32, tag="wores")
                    nc.scalar.activation(tmp[:], psm[:], AF.Identity,
                                         bias=lnp_sb[:, 42 + o:43 + o])
                    nc.vector.tensor_add(X[:, o * T:(o + 1) * T],
                                         X[:, o * T:(o + 1) * T], tmp[:])

            # ---- final LN -> out -----------------------------------------
            nc.sync.dma_start(out=lnp_sb[:, 0:2 * DT], in_=normp_in[:, :])
            xout = nc.alloc_sbuf_tensor("xout", [128, DT * T], F32)
            layernorm(X, 0, xout)
            for t in range(DT):
                nc.sync.dma_start(out=out_t[:, t * T:(t + 1) * T],
                                  in_=xout[:, t * T:(t + 1) * T])

    nc.compile()
    return nc


# ---------------------------------------------------------------- host prep

def _ft_pack(a):
    """[768, T] -> [128, 6*T] ft-tile-major."""
    Tn = a.shape[1]
    return a.reshape(DT, 128, Tn).transpose(1, 0, 2).reshape(128, DT * Tn)


def _pp_pack(v):
    """[n*128] per-feature -> [128, n] per-partition columns."""
    return np.ascontiguousarray(v.reshape(-1, 128).T)


def build_masks(rand_attn):
    """[NH, KPAD(block-tiled 9x128), T] per core r -> mask[r][128, NH*KT*T]."""
    ra = np.asarray(rand_attn)
    # block-level MULTIPLICITY: cnt[h, l, j] = how many times k-block j appears
    # in the reference's concatenated key list for q-block l (duplicated random
    # blocks are counted twice in the reference softmax).
    cnt = np.zeros((NH, NBLK, NBLK), dtype=np.float32)
    cnt[:, 0, :] = 1.0
    cnt[:, 16, :] = 1.0
    for h in range(NH):
        for l in range(1, 16):
            base = {0, 16, l - 1, l, l + 1} if 1 < l < 15 else (
                {0, 1, 2, 16} if l == 1 else {0, 14, 15, 16})
            for j in base:
                cnt[h, l, j] += 1.0
            for r in range(R):
                cnt[h, l, int(ra[h, l - 1, r])] += 1.0
    kvalid = np.zeros((KPAD,), dtype=np.float32)
    kvalid[:SEQ] = 1.0  # tokens 0..1024 real; 1025..1151 invalid
    masks = []
    for r in range(4):
        qg = np.arange(r * T, (r + 1) * T)
        lq = np.minimum(qg // BS, NBLK - 1)
        kg = np.arange(KPAD)
        jk = np.minimum(kg // BS, NBLK - 1)
        m = np.zeros((NH, KPAD, T), dtype=BF)
        for h in range(NH):
            mh = cnt[h].T[np.ix_(jk, lq)] * kvalid[:, None]
            m[h] = mh.astype(BF)
        # -> [128, NH*(KT-1)*T]; k-tile 8 (key 1024) is a constant on device
        m = m.reshape(NH, KT, 128, T)[:, :KT - 1]
        m = m.transpose(2, 0, 1, 3).reshape(128, NH * (KT - 1) * T)
        masks.append(np.ascontiguousarray(m))
    return masks


def prepare_inputs(inputs, nlayers=NLAYERS):
    pv = np.asarray(inputs["pixel_values"], np.float32)
    B = pv.shape[0]
    g_img = pv.shape[2] // 16
    ntok_img = g_img * g_img
    patches = pv.reshape(B, 3, g_img, 16, g_img, 16).transpose(0, 2, 4, 1, 3, 5)
    patches = patches.reshape(B, ntok_img, 768)

    pos = np.asarray(inputs["pos_emb"], np.float32)[0]          # [1025, 768]
    cls = np.asarray(inputs["cls_token"], np.float32).reshape(768)
    patch_b = np.asarray(inputs["patch_b"], np.float32)

    # patchesZ^T [768, 1088] and add_term [768, 1088] per batch
    pzt = np.zeros((B, 768, SEQP), np.float32)
    addt = np.zeros((B, 768, SEQP), np.float32)
    for b in range(B):
        pzt[b, :, 1:1 + ntok_img] = patches[b].T
        addt[b, :, 0] = cls + pos[0]
        addt[b, :, 1:SEQ] = (patch_b[None, :] + pos[1:SEQ]).T

    masks = build_masks(inputs["rand_attn"])

    def bfc(x):
        return np.ascontiguousarray(np.asarray(x, np.float32).astype(BF))

    shared = {"pw": bfc(inputs["patch_w"])}
    normp = np.concatenate(
        [_pp_pack(np.asarray(inputs["norm_g"], np.float32)),
         _pp_pack(np.asarray(inputs["norm_b"], np.float32))], axis=1)
    shared["normp"] = np.ascontiguousarray(normp)
    for i in range(nlayers):
        shared[f"wq{i}"] = bfc(inputs["Wq"][i])
        shared[f"wk{i}"] = bfc(inputs["Wk"][i])
        wva = np.zeros((768, VCOLS), np.float32)
        wv_i = np.asarray(inputs["Wv"][i], np.float32)
        for h in range(NH):
            wva[:, h * 65:h * 65 + 64] = wv_i[:, h * 64:(h + 1) * 64]
        shared[f"wv{i}"] = bfc(wva)
        shared[f"wo{i}"] = bfc(inputs["Wo"][i])
        # o-major tiling: [CT*128, OT*128] -> [128, OT*CT*128]
        w1_i = np.asarray(inputs["ff_w1"][i], np.float32)       # [768, 3072]
        w1t = w1_i.reshape(DT, 128, FT, 128).transpose(1, 2, 0, 3).reshape(128, FT * D)
        shared[f"w1{i}"] = bfc(w1t)
        w2_i = np.asarray(inputs["ff_w2"][i], np.float32)       # [3072, 768]
        w2t = w2_i.reshape(FT, 128, DT, 128).transpose(1, 2, 0, 3).reshape(128, DT * F)
        shared[f"w2{i}"] = bfc(w2t)
        lnp_i = np.zeros((128, 72), np.float32)
        lnp_i[:, 0:6] = _pp_pack(np.asarray(inputs["ln1_g"][i], np.float32))
        lnp_i[:, 6:12] = _pp_pack(np.asarray(inputs["ln1_b"][i], np.float32))
        lnp_i[:, 12:18] = _pp_pack(np.asarray(inputs["ln2_g"][i], np.float32))
        lnp_i[:, 18:24] = _pp_pack(np.asarray(inputs["ln2_b"][i], np.float32))
        lnp_i[:, 24:30] = _pp_pack(np.asarray(inputs["bq"][i], np.float32))
        lnp_i[:, 30:36] = _pp_pack(np.asarray(inputs["bk"][i], np.float32))
        lnp_i[:, 36:42] = _pp_pack(np.asarray(inputs["bo"][i], np.float32))
        lnp_i[:, 42:48] = _pp_pack(np.asarray(inputs["ff_b2"][i], np.float32))
        lnp_i[:, 48:72] = _pp_pack(np.asarray(inputs["ff_b1"][i], np.float32))
        shared[f"lnp{i}"] = np.ascontiguousarray(lnp_i)
        bva = np.zeros((VCOLS,), np.float32)
        bv_i = np.asarray(inputs["bv"][i], np.float32)
        for h in range(NH):
            bva[h * 65:h * 65 + 64] = bv_i[h * 64:(h + 1) * 64]
            bva[h * 65 + 64] = 1.0
        shared[f"bvb{i}"] = np.ascontiguousarray(
            np.broadcast_to(bva.astype(BF), (128, VCOLS)))

    in_maps = []
    for c in range(8):
        g, r = c // 4, c % 4
        im = dict(shared)
        sl = slice(r * T, (r + 1) * T)
        im["pe_in"] = np.ascontiguousarray(_ft_pack(pzt[g][:, sl]).astype(BF))
        im["add_in"] = np.ascontiguousarray(_ft_pack(addt[g][:, sl]))
        im["mask_in"] = masks[r]
        in_maps.append(im)
    return in_maps


LAST_RESULT = None


def kernel(**inputs):
    global LAST_RESULT
    key = ("prog", NLAYERS)
    if key not in _CACHE:
        _CACHE[key] = build_program(NLAYERS)
    nc = _CACHE[key]
    in_maps = prepare_inputs(inputs, NLAYERS)
    kw = {}
    if os.environ.get("BB_TRACE"):
        kw = dict(trace=True, tmpdir=os.environ.get("BB_TRACE_DIR") or None)
    res = run_bass_kernel_spmd(nc, in_maps, core_ids=list(range(8)), **kw)
    LAST_RESULT = res
    outs = []
    for g in range(2):
        cols = []
        for r in range(4):
            o = res.results[g * 4 + r]["out"]          # [128, 6*T]
            o = o.reshape(128, DT, T).transpose(1, 0, 2).reshape(768, T)
            cols.append(o)
        xt = np.concatenate(cols, axis=1)              # [768, 1088]
        outs.append(xt[:, :SEQ].T)                     # [1025, 768]
    return np.stack(outs, axis=0).astype(np.float32)


if __name__ == "__main__":
    import reference
    ins = {k: np.asarray(v) for k, v in reference.setup_inputs().items()}
    got = kernel(**ins)
    print("kernel output", got.shape)

